# revision 9
# baseline (speedup 1.0000x reference)
"""Multi-class 3D DICE loss on 8 Trainium2 NeuronCores.

Data-parallel over the subject (batch) axis: core b reduces subject b's
[C=4, 64, 128, 128] volumes to per-class (inter, mask_sum, out_sum) partial
sums; the host applies the ~10-flop DICE scalar tail and averages the 8
per-subject losses.

Per-core layout: each input tensor is viewed as [128, 32768] where
partition q = c*32 + p (class c in partition block [32c, 32c+32)).

Engine split per chunk: DVE scalar_tensor_tensor (inter partials), ACT
activation-Copy accum (x and m partials), PE collapses partition blocks
into per-class sums with one matmul at the end. DVE ~1 pass and ACT ~2
passes both stay ahead of the ~80 us DMA stream.

SDMA straggler mitigation: HWDGE splits an R-row transfer into R/b blocks
(b = smallest divisor of R that is >= R/16), assigned to engines 0..R/b-1;
engine 15 is intermittently ~18% slower (observed 21.6 vs 26.3 GB/s, 4/5
runs). So 6144 of the 32768 columns are moved with [0:120)-row chunks
(15 engines x 8 rows -> engine 15 idle) plus 16 one-row flat DMAs for
rows 120-127 (flats spray evenly across all 16 engines). That leaves
engine 15 with 26624 cols x 8 rows ~= 0.82x the per-engine average --
matching its slow rate, so all 16 engines finish together. The 120-row
chunks are interleaved between full chunks so the deficit never
accumulates, and the stream ENDS on a 120-row chunk so the final
completion never waits on engine 15.
"""

import os
import sys
from contextlib import ExitStack

import numpy as np

for _p in ("/opt/trn_rl_repo",):
    if _p not in sys.path and os.path.isdir(_p):
        sys.path.insert(0, _p)

import concourse.bass as bass  # noqa: E402
import concourse.tile as tile  # noqa: E402
from concourse import bacc, mybir  # noqa: E402
from concourse.bass_utils import run_bass_kernel_spmd  # noqa: E402

N_CORES = 8
B, C = 8, 4
SPATIAL = 64 * 128 * 128            # 1,048,576 per (subject, class)
P = 128                             # SBUF partitions = C * 32
COLS = (C * SPATIAL) // P           # 32768 elements per partition
EPS = 1e-7
F32 = mybir.dt.float32

# Full-height chunks (all 128 rows; engine 15 gets 8 rows of each).
FULL = [(0, 4096), (4096, 6144), (10240, 6144), (16384, 6144), (22528, 4096)]
# 120-row chunks (engines 0-14 only), descending so the compute tail is tiny.
SHED0, SHED_FD = 26624, 6144
T120 = [(26624, 3072), (29696, 2048), (31744, 768), (32512, 256)]
assert FULL[-1][0] + FULL[-1][1] == SHED0
assert T120[-1][0] + T120[-1][1] == COLS
assert sum(fd for _, fd in T120) == SHED_FD

# Accumulator column layout ([P, 40]), one column per compute chunk below.
# SBUF accumulator words (32 B = 8 fp32 cols) must not see concurrent
# writes from two engines; the single ordered memset at t0 plus
# one-engine-per-(row, word) accumulation keeps every word safe.
#   cols  0..8   inter, DVE         cols 16..23  msum chunks 0..7, ACT
#   col   10     shed inter, DVE    cols 24..32  xsum, ACT
#   col   15     msum last, DVE     col 34/35    shed msum/xsum, ACT
# The shed tile lives on partitions 0-7 (ACT cannot start at partition
# 120), so its sums surface in the class-0 row of the collapse; the host
# adds them back into class 3.
NCHUNK = 9
SHED_INTER, SHED_MSUM, SHED_XSUM = 10, 34, 35
MSUM_DVE = 15
MSUM0 = 16
XSUM0 = 24
ACC_COLS = 40
LAST = NCHUNK - 1


def _dice_body(ctx: ExitStack, tc: "tile.TileContext", out_ap, x_ap, m_ap):
    nc = tc.nc
    add = mybir.AluOpType.add
    mult = mybir.AluOpType.mult
    Copy = mybir.ActivationFunctionType.Copy

    consts = ctx.enter_context(tc.tile_pool(name="consts", bufs=1))
    xpool = ctx.enter_context(tc.tile_pool(name="xin", bufs=2))
    mpool = ctx.enter_context(tc.tile_pool(name="min", bufs=2))
    xt120 = ctx.enter_context(tc.tile_pool(name="xt120", bufs=2))
    mt120 = ctx.enter_context(tc.tile_pool(name="mt120", bufs=2))
    shedp = ctx.enter_context(tc.tile_pool(name="shed", bufs=1))
    small = ctx.enter_context(tc.tile_pool(name="small", bufs=1))
    psum = ctx.enter_context(tc.tile_pool(name="psum", bufs=1, space="PSUM"))

    # Block indicator: ind[q, c] = 1.0 iff q // 32 == c. lhsT for the
    # partition-block -> per-class collapse.
    ind = consts.tile([P, C], F32)
    nc.vector.memset(ind[:], 0.0)
    for c in range(C):
        nc.vector.memset(ind[c * 32 : (c + 1) * 32, c : c + 1], 1.0)

    # Per-chunk partial sums. Zeroed once so partial-height chunks leave
    # their uncovered rows (and pad columns) at exactly 0.
    acc = small.tile([P, ACC_COLS], F32)
    nc.vector.memset(acc[:], 0.0)
    # Engines must write their full elementwise result somewhere; stride-0
    # broadcast dummies avoid real [P, fd] scratch tiles (HW-verified).
    dve_dummy = small.tile([P, 1], F32)
    act_dummy = small.tile([P, 1], F32)
    act_dummy2 = small.tile([P, 1], F32)

    def compute(xt, mt, n, cols):
        """Partial sums over tiles xt, mt rows [0:n) into acc cols
        (inter, xsum, msum) — msum None means DVE tensor_reduce into
        MSUM_DVE (used for the final chunk so ACT and DVE finish in
        parallel right after the last bytes land)."""
        fd = xt.shape[-1]
        c_inter, c_xsum, c_msum = cols
        nc.vector.scalar_tensor_tensor(
            out=dve_dummy[0:n, 0:1].broadcast_to((n, fd)),
            in0=xt[0:n, :],
            scalar=1.0,
            in1=mt[0:n, :],
            op0=mult,
            op1=mult,
            accum_out=acc[0:n, c_inter : c_inter + 1],
        )
        nc.scalar.activation(
            out=act_dummy2[0:n, 0:1].broadcast_to((n, fd)),
            in_=xt[0:n, :],
            func=Copy,
            accum_out=acc[0:n, c_xsum : c_xsum + 1],
        )
        if c_msum is not None:
            nc.scalar.activation(
                out=act_dummy[0:n, 0:1].broadcast_to((n, fd)),
                in_=mt[0:n, :],
                func=Copy,
                accum_out=acc[0:n, c_msum : c_msum + 1],
            )
        else:
            nc.vector.tensor_reduce(
                acc[0:n, MSUM_DVE : MSUM_DVE + 1],
                mt[0:n, :],
                axis=mybir.AxisListType.X,
                op=add,
            )

    def chunk_cols(k):
        return (k, XSUM0 + k, MSUM0 + k if k < LAST else None)

    def full_chunk(off, fd, k):
        xt = xpool.tile([P, fd], F32, tag="xt")
        nc.sync.dma_start(out=xt[:], in_=x_ap[:, off : off + fd])
        mt = mpool.tile([P, fd], F32, tag="mt")
        nc.sync.dma_start(out=mt[:], in_=m_ap[:, off : off + fd])
        compute(xt, mt, P, chunk_cols(k))

    def t120_chunk(off, fd, k):
        xt = xt120.tile([120, fd], F32, tag="xt")
        nc.sync.dma_start(out=xt[:], in_=x_ap[0:120, off : off + fd])
        mt = mt120.tile([120, fd], F32, tag="mt")
        nc.sync.dma_start(out=mt[:], in_=m_ap[0:120, off : off + fd])
        compute(xt, mt, 120, chunk_cols(k))

    # Interleave so engine 15's per-chunk deficit never accumulates and the
    # stream ends on a 120-row chunk (ring order == issue order below).
    xs = shedp.tile([8, SHED_FD], F32, tag="xs")
    ms = shedp.tile([8, SHED_FD], F32, tag="ms")

    full_chunk(*FULL[0], 0)
    t120_chunk(*T120[0], 1)
    full_chunk(*FULL[1], 2)
    # Rows 120-127 of the shed columns: 16 one-row flat DMAs, each sprayed
    # evenly across all 16 SDMA engines. They land on shed-tile rows 0-7.
    for p in range(120, 128):
        i = p - 120
        nc.sync.dma_start(out=xs[i : i + 1, :], in_=x_ap[p : p + 1, SHED0:COLS])
        nc.sync.dma_start(out=ms[i : i + 1, :], in_=m_ap[p : p + 1, SHED0:COLS])
    compute(xs, ms, 8, (SHED_INTER, SHED_XSUM, SHED_MSUM))
    t120_chunk(*T120[1], 3)
    full_chunk(*FULL[2], 4)
    full_chunk(*FULL[3], 5)
    t120_chunk(*T120[2], 6)
    full_chunk(*FULL[4], 7)
    t120_chunk(*T120[3], 8)

    # Partition blocks -> per-(class, chunk) sums in one matmul, then three
    # exact-range PSUM reduces -> [4, 3] class sums (inter, msum, xsum).
    # The remaining ~10-flop scalar tail runs on the host during unshard.
    ps = psum.tile([C, ACC_COLS], F32)
    nc.tensor.matmul(out=ps[:], lhsT=ind[:], rhs=acc[:], start=True, stop=True)
    sums = small.tile([C, 6], F32)
    nc.vector.tensor_reduce(
        sums[:, 0:1], ps[:, 0:NCHUNK], axis=mybir.AxisListType.X, op=add
    )
    nc.vector.tensor_reduce(
        sums[:, 1:2], ps[:, MSUM_DVE : MSUM0 + LAST], axis=mybir.AxisListType.X, op=add
    )
    nc.vector.tensor_reduce(
        sums[:, 2:3], ps[:, XSUM0 : XSUM0 + NCHUNK], axis=mybir.AxisListType.X, op=add
    )
    # Shed sums (surface in the class-0 row; host adds them to class 3).
    for dst, src_col in ((3, SHED_INTER), (4, SHED_MSUM), (5, SHED_XSUM)):
        nc.vector.tensor_reduce(
            sums[:, dst : dst + 1],
            ps[:, src_col : src_col + 1],
            axis=mybir.AxisListType.X,
            op=add,
        )
    nc.sync.dma_start(out=out_ap, in_=sums[:])


_CACHE: dict[str, object] = {}


def _build():
    if "nc" in _CACHE:
        return _CACHE["nc"]
    nc = bacc.Bacc("TRN2", target_bir_lowering=False, debug=False)
    x = nc.dram_tensor("x", [P, COLS], F32, kind="ExternalInput").ap()
    m = nc.dram_tensor("m", [P, COLS], F32, kind="ExternalInput").ap()
    out = nc.dram_tensor("class_sums", [C, 6], F32, kind="ExternalOutput").ap()
    with tile.TileContext(nc) as tc:
        with ExitStack() as ctx:
            _dice_body(ctx, tc, out, x, m)
    nc.compile()
    _CACHE["nc"] = nc
    return nc


def _in_maps(output: np.ndarray, masks: np.ndarray):
    output = np.ascontiguousarray(output, dtype=np.float32)
    masks = np.ascontiguousarray(masks, dtype=np.float32)
    return [
        {"x": output[b].reshape(P, COLS), "m": masks[b].reshape(P, COLS)}
        for b in range(N_CORES)
    ]


def _finish(cs: np.ndarray) -> np.float32:
    """Per-subject scalar tail (fp32, mirrors the reference ordering).

    cs: [C, 6] device output — columns (inter, mask_sum, x_sum) per class,
    then (inter, mask_sum, x_sum) of the shed block (class 3, surfaced in
    row 0 by the collapse).
    """
    cs = cs.astype(np.float32)
    inter, msum, xsum = cs[:, 0].copy(), cs[:, 1].copy(), cs[:, 2].copy()
    inter[3] += cs[0, 3]
    msum[3] += cs[0, 4]
    xsum[3] += cs[0, 5]
    w = np.float32(1.0) / (msum * msum + np.float32(EPS))
    total = xsum + msum
    nom = (w * inter).sum(dtype=np.float32)
    den = (w * total + np.float32(EPS)).sum(dtype=np.float32)
    return np.float32(1.0) - np.float32(2.0) * nom / den


def run_sharded(output: np.ndarray, masks: np.ndarray, **spmd_kwargs):
    """Run the SPMD kernel; returns (loss[1], BassKernelResults)."""
    nc = _build()
    res = run_bass_kernel_spmd(
        nc, _in_maps(output, masks), list(range(N_CORES)), **spmd_kwargs
    )
    per_subj = np.array(
        [_finish(res.results[b]["class_sums"]) for b in range(N_CORES)],
        dtype=np.float32,
    )
    loss = (per_subj.sum(dtype=np.float32) / np.float32(B)).reshape(1)
    return loss.astype(np.float32), res


def kernel(output: np.ndarray, masks: np.ndarray) -> np.ndarray:
    loss, _ = run_sharded(output, masks)
    return loss


# revision 10
# speedup vs baseline: 1.0956x; 1.0956x over previous
"""Multi-class 3D DICE loss on 8 Trainium2 NeuronCores.

Data-parallel over the subject (batch) axis: core b reduces subject b's
[C=4, 64, 128, 128] volumes to per-class (inter, mask_sum, out_sum) partial
sums; the host applies the ~10-flop DICE scalar tail and averages the 8
per-subject losses.

Per-core layout: each input tensor is viewed as [128, 32768] where
partition q = c*32 + p (class c in partition block [32c, 32c+32)).

Engine split per chunk: DVE scalar_tensor_tensor (inter partials), ACT
activation-Copy accum (x and m partials), PE collapses partition blocks
into per-class sums with one matmul at the end. DVE ~1 pass and ACT ~2
passes both stay ahead of the ~80 us DMA stream.

SDMA straggler mitigation: HWDGE splits an R-row transfer into R/b blocks
(b = smallest divisor of R that is >= R/16), assigned to engines 0..R/b-1;
engine 15 is intermittently ~18% slower (observed 21.6 vs 26.3 GB/s, 4/5
runs). So 6144 of the 32768 columns are moved with [0:120)-row chunks
(15 engines x 8 rows -> engine 15 idle) plus 16 one-row flat DMAs for
rows 120-127 (flats spray evenly across all 16 engines). That leaves
engine 15 with 26624 cols x 8 rows ~= 0.82x the per-engine average --
matching its slow rate, so all 16 engines finish together. The 120-row
chunks are interleaved between full chunks so the deficit never
accumulates, and the stream ENDS on a 120-row chunk so the final
completion never waits on engine 15.
"""

import os
import sys
from contextlib import ExitStack

import numpy as np

for _p in ("/opt/trn_rl_repo",):
    if _p not in sys.path and os.path.isdir(_p):
        sys.path.insert(0, _p)

import concourse.bass as bass  # noqa: E402
import concourse.tile as tile  # noqa: E402
from concourse import bacc, mybir  # noqa: E402
from concourse.bass_utils import run_bass_kernel_spmd  # noqa: E402

N_CORES = 8
B, C = 8, 4
SPATIAL = 64 * 128 * 128            # 1,048,576 per (subject, class)
P = 128                             # SBUF partitions = C * 32
COLS = (C * SPATIAL) // P           # 32768 elements per partition
EPS = 1e-7
F32 = mybir.dt.float32

# Full-height chunks (all 128 rows; engine 15 gets 8 rows of each).
FULL = [(0, 4096), (4096, 6144), (10240, 6144), (16384, 6144), (22528, 4096)]
# 120-row chunks (engines 0-14 only), descending so the compute tail is tiny.
SHED0, SHED_FD = 26624, 6144
T120 = [(26624, 3072), (29696, 2048), (31744, 768), (32512, 256)]
assert FULL[-1][0] + FULL[-1][1] == SHED0
assert T120[-1][0] + T120[-1][1] == COLS
assert sum(fd for _, fd in T120) == SHED_FD

# Accumulator column layout ([P, 40]), one column per compute chunk below.
# SBUF accumulator words (32 B = 8 fp32 cols) must not see concurrent
# writes from two engines; the single ordered memset at t0 plus
# one-engine-per-(row, word) accumulation keeps every word safe.
#   cols  0..8   inter, DVE         cols 16..23  msum chunks 0..7, ACT
#   col   10     shed inter, DVE    cols 24..32  xsum, ACT
#   col   15     msum last, DVE     col 34/35    shed msum/xsum, ACT
# The shed tile lives on partitions 0-7 (ACT cannot start at partition
# 120), so its sums surface in the class-0 row of the collapse; the host
# adds them back into class 3.
NCHUNK = 9
SHED_INTER, SHED_MSUM, SHED_XSUM = 10, 34, 35
MSUM_DVE = 15
MSUM0 = 16
XSUM0 = 24
ACC_COLS = 40
LAST = NCHUNK - 1


def _dice_body(ctx: ExitStack, tc: "tile.TileContext", out_ap, x_ap, m_ap):
    nc = tc.nc
    add = mybir.AluOpType.add
    mult = mybir.AluOpType.mult
    Copy = mybir.ActivationFunctionType.Copy

    consts = ctx.enter_context(tc.tile_pool(name="consts", bufs=1))
    xpool = ctx.enter_context(tc.tile_pool(name="xin", bufs=2))
    mpool = ctx.enter_context(tc.tile_pool(name="min", bufs=2))
    xt120 = ctx.enter_context(tc.tile_pool(name="xt120", bufs=2))
    mt120 = ctx.enter_context(tc.tile_pool(name="mt120", bufs=2))
    shedp = ctx.enter_context(tc.tile_pool(name="shed", bufs=1))
    small = ctx.enter_context(tc.tile_pool(name="small", bufs=1))
    psum = ctx.enter_context(tc.tile_pool(name="psum", bufs=1, space="PSUM"))

    # Block indicator: ind[q, c] = 1.0 iff q // 32 == c. lhsT for the
    # partition-block -> per-class collapse.
    ind = consts.tile([P, C], F32)
    nc.vector.memset(ind[:], 0.0)
    for c in range(C):
        nc.vector.memset(ind[c * 32 : (c + 1) * 32, c : c + 1], 1.0)

    # Per-chunk partial sums. Zeroed once so partial-height chunks leave
    # their uncovered rows (and pad columns) at exactly 0.
    acc = small.tile([P, ACC_COLS], F32)
    nc.vector.memset(acc[:], 0.0)
    # Engines must write their full elementwise result somewhere; stride-0
    # broadcast dummies avoid real [P, fd] scratch tiles (HW-verified).
    dve_dummy = small.tile([P, 1], F32)
    act_dummy = small.tile([P, 1], F32)
    act_dummy2 = small.tile([P, 1], F32)

    def compute(xt, mt, n, cols):
        """Partial sums over tiles xt, mt rows [0:n) into acc cols
        (inter, xsum, msum) — msum None means DVE tensor_reduce into
        MSUM_DVE (used for the final chunk so ACT and DVE finish in
        parallel right after the last bytes land)."""
        fd = xt.shape[-1]
        c_inter, c_xsum, c_msum = cols
        nc.vector.scalar_tensor_tensor(
            out=dve_dummy[0:n, 0:1].broadcast_to((n, fd)),
            in0=xt[0:n, :],
            scalar=1.0,
            in1=mt[0:n, :],
            op0=mult,
            op1=mult,
            accum_out=acc[0:n, c_inter : c_inter + 1],
        )
        nc.scalar.activation(
            out=act_dummy2[0:n, 0:1].broadcast_to((n, fd)),
            in_=xt[0:n, :],
            func=Copy,
            accum_out=acc[0:n, c_xsum : c_xsum + 1],
        )
        if c_msum is not None:
            nc.scalar.activation(
                out=act_dummy[0:n, 0:1].broadcast_to((n, fd)),
                in_=mt[0:n, :],
                func=Copy,
                accum_out=acc[0:n, c_msum : c_msum + 1],
            )
        else:
            nc.vector.tensor_reduce(
                acc[0:n, MSUM_DVE : MSUM_DVE + 1],
                mt[0:n, :],
                axis=mybir.AxisListType.X,
                op=add,
            )

    def chunk_cols(k):
        return (k, XSUM0 + k, MSUM0 + k if k < LAST else None)

    def full_chunk(off, fd, k):
        xt = xpool.tile([P, fd], F32, tag="xt")
        nc.sync.dma_start(out=xt[:], in_=x_ap[:, off : off + fd])
        mt = mpool.tile([P, fd], F32, tag="mt")
        nc.sync.dma_start(out=mt[:], in_=m_ap[:, off : off + fd])
        compute(xt, mt, P, chunk_cols(k))

    def t120_chunk(off, fd, k):
        xt = xt120.tile([120, fd], F32, tag="xt")
        nc.sync.dma_start(out=xt[:], in_=x_ap[0:120, off : off + fd])
        mt = mt120.tile([120, fd], F32, tag="mt")
        nc.sync.dma_start(out=mt[:], in_=m_ap[0:120, off : off + fd])
        compute(xt, mt, 120, chunk_cols(k))

    # Interleave so engine 15's per-chunk deficit never accumulates and the
    # stream ends on a 120-row chunk (ring order == issue order below).
    xs = shedp.tile([8, SHED_FD], F32, tag="xs")
    ms = shedp.tile([8, SHED_FD], F32, tag="ms")

    full_chunk(*FULL[0], 0)
    t120_chunk(*T120[0], 1)
    full_chunk(*FULL[1], 2)
    # Rows 120-127 of the shed columns: one 8-row DMA per tensor (8 rows
    # -> engines 0-7, one 24 KB row each). They land on shed-tile rows 0-7.
    nc.sync.dma_start(out=xs[:], in_=x_ap[120:128, SHED0:COLS])
    nc.sync.dma_start(out=ms[:], in_=m_ap[120:128, SHED0:COLS])
    compute(xs, ms, 8, (SHED_INTER, SHED_XSUM, SHED_MSUM))
    t120_chunk(*T120[1], 3)
    full_chunk(*FULL[2], 4)
    full_chunk(*FULL[3], 5)
    t120_chunk(*T120[2], 6)
    full_chunk(*FULL[4], 7)
    t120_chunk(*T120[3], 8)

    # Partition blocks -> per-(class, chunk) sums in one matmul, then three
    # exact-range PSUM reduces -> [4, 3] class sums (inter, msum, xsum).
    # The remaining ~10-flop scalar tail runs on the host during unshard.
    ps = psum.tile([C, ACC_COLS], F32)
    nc.tensor.matmul(out=ps[:], lhsT=ind[:], rhs=acc[:], start=True, stop=True)
    sums = small.tile([C, 6], F32)
    nc.vector.tensor_reduce(
        sums[:, 0:1], ps[:, 0:NCHUNK], axis=mybir.AxisListType.X, op=add
    )
    nc.vector.tensor_reduce(
        sums[:, 1:2], ps[:, MSUM_DVE : MSUM0 + LAST], axis=mybir.AxisListType.X, op=add
    )
    nc.vector.tensor_reduce(
        sums[:, 2:3], ps[:, XSUM0 : XSUM0 + NCHUNK], axis=mybir.AxisListType.X, op=add
    )
    # Shed sums (surface in the class-0 row; host adds them to class 3).
    for dst, src_col in ((3, SHED_INTER), (4, SHED_MSUM), (5, SHED_XSUM)):
        nc.vector.tensor_reduce(
            sums[:, dst : dst + 1],
            ps[:, src_col : src_col + 1],
            axis=mybir.AxisListType.X,
            op=add,
        )
    nc.sync.dma_start(out=out_ap, in_=sums[:])


_CACHE: dict[str, object] = {}


def _build():
    if "nc" in _CACHE:
        return _CACHE["nc"]
    nc = bacc.Bacc("TRN2", target_bir_lowering=False, debug=False)
    x = nc.dram_tensor("x", [P, COLS], F32, kind="ExternalInput").ap()
    m = nc.dram_tensor("m", [P, COLS], F32, kind="ExternalInput").ap()
    out = nc.dram_tensor("class_sums", [C, 6], F32, kind="ExternalOutput").ap()
    with tile.TileContext(nc) as tc:
        with ExitStack() as ctx:
            _dice_body(ctx, tc, out, x, m)
    nc.compile()
    _CACHE["nc"] = nc
    return nc


def _in_maps(output: np.ndarray, masks: np.ndarray):
    output = np.ascontiguousarray(output, dtype=np.float32)
    masks = np.ascontiguousarray(masks, dtype=np.float32)
    return [
        {"x": output[b].reshape(P, COLS), "m": masks[b].reshape(P, COLS)}
        for b in range(N_CORES)
    ]


def _finish(cs: np.ndarray) -> np.float32:
    """Per-subject scalar tail (fp32, mirrors the reference ordering).

    cs: [C, 6] device output — columns (inter, mask_sum, x_sum) per class,
    then (inter, mask_sum, x_sum) of the shed block (class 3, surfaced in
    row 0 by the collapse).
    """
    cs = cs.astype(np.float32)
    inter, msum, xsum = cs[:, 0].copy(), cs[:, 1].copy(), cs[:, 2].copy()
    inter[3] += cs[0, 3]
    msum[3] += cs[0, 4]
    xsum[3] += cs[0, 5]
    w = np.float32(1.0) / (msum * msum + np.float32(EPS))
    total = xsum + msum
    nom = (w * inter).sum(dtype=np.float32)
    den = (w * total + np.float32(EPS)).sum(dtype=np.float32)
    return np.float32(1.0) - np.float32(2.0) * nom / den


def run_sharded(output: np.ndarray, masks: np.ndarray, **spmd_kwargs):
    """Run the SPMD kernel; returns (loss[1], BassKernelResults)."""
    nc = _build()
    res = run_bass_kernel_spmd(
        nc, _in_maps(output, masks), list(range(N_CORES)), **spmd_kwargs
    )
    per_subj = np.array(
        [_finish(res.results[b]["class_sums"]) for b in range(N_CORES)],
        dtype=np.float32,
    )
    loss = (per_subj.sum(dtype=np.float32) / np.float32(B)).reshape(1)
    return loss.astype(np.float32), res


def kernel(output: np.ndarray, masks: np.ndarray) -> np.ndarray:
    loss, _ = run_sharded(output, masks)
    return loss


# revision 11
# speedup vs baseline: 1.1508x; 1.0504x over previous
"""Multi-class 3D DICE loss on 8 Trainium2 NeuronCores.

Data-parallel over the subject (batch) axis: core b reduces subject b's
[C=4, 64, 128, 128] volumes to per-class (inter, mask_sum, out_sum) partial
sums; the host applies the ~10-flop DICE scalar tail and averages the 8
per-subject losses.

Per-core layout: each input tensor is viewed as [128, 32768] where
partition q = c*32 + p (class c in partition block [32c, 32c+32)).

Engine split per chunk: DVE scalar_tensor_tensor (inter partials), ACT
activation-Copy accum (x and m partials), PE collapses partition blocks
into per-class sums with one matmul at the end. DVE ~1 pass and ACT ~2
passes both stay ahead of the ~80 us DMA stream.

SDMA straggler mitigation: HWDGE splits an R-row transfer into R/b blocks
(b = smallest divisor of R that is >= R/16), assigned to engines 0..R/b-1;
engine 15 is intermittently ~18% slower (observed 21.6 vs 26.3 GB/s, 4/5
runs). So 6144 of the 32768 columns are moved with [0:120)-row chunks
(15 engines x 8 rows -> engine 15 idle) plus 16 one-row flat DMAs for
rows 120-127 (flats spray evenly across all 16 engines). That leaves
engine 15 with 26624 cols x 8 rows ~= 0.82x the per-engine average --
matching its slow rate, so all 16 engines finish together. The 120-row
chunks are interleaved between full chunks so the deficit never
accumulates, and the stream ENDS on a 120-row chunk so the final
completion never waits on engine 15.
"""

import os
import sys
from contextlib import ExitStack

import numpy as np

for _p in ("/opt/trn_rl_repo",):
    if _p not in sys.path and os.path.isdir(_p):
        sys.path.insert(0, _p)

import concourse.bass as bass  # noqa: E402
import concourse.tile as tile  # noqa: E402
from concourse import bacc, mybir  # noqa: E402
from concourse.bass_utils import run_bass_kernel_spmd  # noqa: E402

N_CORES = 8
B, C = 8, 4
SPATIAL = 64 * 128 * 128            # 1,048,576 per (subject, class)
P = 128                             # SBUF partitions = C * 32
COLS = (C * SPATIAL) // P           # 32768 elements per partition
EPS = 1e-7
F32 = mybir.dt.float32

# Full-height chunks (all 128 rows; engine 15 gets 8 rows of each).
FULL = [(0, 6144), (6144, 6144), (12288, 6144), (18432, 4096), (22528, 4096)]
# 120-row chunks (engines 0-14 only), descending so the compute tail is tiny.
SHED0, SHED_FD = 26624, 6144
T120 = [(26624, 3072), (29696, 2048), (31744, 768), (32512, 256)]
assert FULL[-1][0] + FULL[-1][1] == SHED0
assert T120[-1][0] + T120[-1][1] == COLS
assert sum(fd for _, fd in T120) == SHED_FD

# Accumulator column layout ([P, 40]), one column per compute chunk below.
# SBUF accumulator words (32 B = 8 fp32 cols) must not see concurrent
# writes from two engines; the single ordered memset at t0 plus
# one-engine-per-(row, word) accumulation keeps every word safe.
#   cols  0..8   inter, DVE         cols 16..23  msum chunks 0..7, ACT
#   col   10     shed inter, DVE    cols 24..32  xsum, ACT
#   col   15     msum last, DVE     col 34/35    shed msum/xsum, ACT
# The shed tile lives on partitions 0-7 (ACT cannot start at partition
# 120), so its sums surface in the class-0 row of the collapse; the host
# adds them back into class 3.
NCHUNK = 9
SHED_INTER, SHED_MSUM, SHED_XSUM = 10, 34, 35
MSUM_DVE = 15
MSUM0 = 16
XSUM0 = 24
ACC_COLS = 40
LAST = NCHUNK - 1


def _dice_body(ctx: ExitStack, tc: "tile.TileContext", out_ap, x_ap, m_ap):
    nc = tc.nc
    add = mybir.AluOpType.add
    mult = mybir.AluOpType.mult
    Copy = mybir.ActivationFunctionType.Copy

    consts = ctx.enter_context(tc.tile_pool(name="consts", bufs=1))
    xpool = ctx.enter_context(tc.tile_pool(name="xin", bufs=2))
    mpool = ctx.enter_context(tc.tile_pool(name="min", bufs=2))
    xt120 = ctx.enter_context(tc.tile_pool(name="xt120", bufs=2))
    mt120 = ctx.enter_context(tc.tile_pool(name="mt120", bufs=2))
    shedp = ctx.enter_context(tc.tile_pool(name="shed", bufs=1))
    small = ctx.enter_context(tc.tile_pool(name="small", bufs=1))
    psum = ctx.enter_context(tc.tile_pool(name="psum", bufs=1, space="PSUM"))

    # Block indicator: ind[q, c] = 1.0 iff q // 32 == c. lhsT for the
    # partition-block -> per-class collapse.
    ind = consts.tile([P, C], F32)
    nc.vector.memset(ind[:], 0.0)
    for c in range(C):
        nc.vector.memset(ind[c * 32 : (c + 1) * 32, c : c + 1], 1.0)

    # Per-chunk partial sums. Zeroed once so partial-height chunks leave
    # their uncovered rows (and pad columns) at exactly 0.
    acc = small.tile([P, ACC_COLS], F32)
    nc.vector.memset(acc[:], 0.0)
    # Engines must write their full elementwise result somewhere; stride-0
    # broadcast dummies avoid real [P, fd] scratch tiles (HW-verified).
    dve_dummy = small.tile([P, 1], F32)
    act_dummy = small.tile([P, 1], F32)
    act_dummy2 = small.tile([P, 1], F32)

    def compute(xt, mt, n, cols):
        """Partial sums over tiles xt, mt rows [0:n) into acc cols
        (inter, xsum, msum) — msum None means DVE tensor_reduce into
        MSUM_DVE (used for the final chunk so ACT and DVE finish in
        parallel right after the last bytes land)."""
        fd = xt.shape[-1]
        c_inter, c_xsum, c_msum = cols
        nc.vector.scalar_tensor_tensor(
            out=dve_dummy[0:n, 0:1].broadcast_to((n, fd)),
            in0=xt[0:n, :],
            scalar=1.0,
            in1=mt[0:n, :],
            op0=mult,
            op1=mult,
            accum_out=acc[0:n, c_inter : c_inter + 1],
        )
        nc.scalar.activation(
            out=act_dummy2[0:n, 0:1].broadcast_to((n, fd)),
            in_=xt[0:n, :],
            func=Copy,
            accum_out=acc[0:n, c_xsum : c_xsum + 1],
        )
        if c_msum is not None:
            nc.scalar.activation(
                out=act_dummy[0:n, 0:1].broadcast_to((n, fd)),
                in_=mt[0:n, :],
                func=Copy,
                accum_out=acc[0:n, c_msum : c_msum + 1],
            )
        else:
            nc.vector.tensor_reduce(
                acc[0:n, MSUM_DVE : MSUM_DVE + 1],
                mt[0:n, :],
                axis=mybir.AxisListType.X,
                op=add,
            )

    def chunk_cols(k):
        return (k, XSUM0 + k, MSUM0 + k if k < LAST else None)

    def full_chunk(off, fd, k):
        xt = xpool.tile([P, fd], F32, tag="xt")
        nc.sync.dma_start(out=xt[:], in_=x_ap[:, off : off + fd])
        mt = mpool.tile([P, fd], F32, tag="mt")
        nc.sync.dma_start(out=mt[:], in_=m_ap[:, off : off + fd])
        compute(xt, mt, P, chunk_cols(k))

    def t120_chunk(off, fd, k):
        xt = xt120.tile([120, fd], F32, tag="xt")
        nc.sync.dma_start(out=xt[:], in_=x_ap[0:120, off : off + fd])
        mt = mt120.tile([120, fd], F32, tag="mt")
        nc.sync.dma_start(out=mt[:], in_=m_ap[0:120, off : off + fd])
        compute(xt, mt, 120, chunk_cols(k))

    # Interleave so engine 15's per-chunk deficit never accumulates and the
    # stream ends on a 120-row chunk (ring order == issue order below).
    xs = shedp.tile([8, SHED_FD], F32, tag="xs")
    ms = shedp.tile([8, SHED_FD], F32, tag="ms")

    full_chunk(*FULL[0], 0)
    full_chunk(*FULL[1], 1)
    t120_chunk(*T120[0], 2)
    full_chunk(*FULL[2], 3)
    t120_chunk(*T120[1], 4)
    # Rows 120-127 of the shed columns: one 8-row DMA per tensor (8 rows
    # -> engines 0-7, one 24 KB row each). They land on shed-tile rows 0-7.
    nc.sync.dma_start(out=xs[:], in_=x_ap[120:128, SHED0:COLS])
    nc.sync.dma_start(out=ms[:], in_=m_ap[120:128, SHED0:COLS])
    compute(xs, ms, 8, (SHED_INTER, SHED_XSUM, SHED_MSUM))
    full_chunk(*FULL[3], 5)
    t120_chunk(*T120[2], 6)
    full_chunk(*FULL[4], 7)
    t120_chunk(*T120[3], 8)

    # Partition blocks -> per-(class, chunk) sums in one matmul, then three
    # exact-range PSUM reduces -> [4, 3] class sums (inter, msum, xsum).
    # The remaining ~10-flop scalar tail runs on the host during unshard.
    ps = psum.tile([C, ACC_COLS], F32)
    nc.tensor.matmul(out=ps[:], lhsT=ind[:], rhs=acc[:], start=True, stop=True)
    sums = small.tile([C, 6], F32)
    nc.vector.tensor_reduce(
        sums[:, 0:1], ps[:, 0:NCHUNK], axis=mybir.AxisListType.X, op=add
    )
    nc.vector.tensor_reduce(
        sums[:, 1:2], ps[:, MSUM_DVE : MSUM0 + LAST], axis=mybir.AxisListType.X, op=add
    )
    nc.vector.tensor_reduce(
        sums[:, 2:3], ps[:, XSUM0 : XSUM0 + NCHUNK], axis=mybir.AxisListType.X, op=add
    )
    # Shed sums (surface in the class-0 row; host adds them to class 3).
    for dst, src_col in ((3, SHED_INTER), (4, SHED_MSUM), (5, SHED_XSUM)):
        nc.vector.tensor_reduce(
            sums[:, dst : dst + 1],
            ps[:, src_col : src_col + 1],
            axis=mybir.AxisListType.X,
            op=add,
        )
    nc.sync.dma_start(out=out_ap, in_=sums[:])


_CACHE: dict[str, object] = {}


def _build():
    if "nc" in _CACHE:
        return _CACHE["nc"]
    nc = bacc.Bacc("TRN2", target_bir_lowering=False, debug=False)
    x = nc.dram_tensor("x", [P, COLS], F32, kind="ExternalInput").ap()
    m = nc.dram_tensor("m", [P, COLS], F32, kind="ExternalInput").ap()
    out = nc.dram_tensor("class_sums", [C, 6], F32, kind="ExternalOutput").ap()
    with tile.TileContext(nc) as tc:
        with ExitStack() as ctx:
            _dice_body(ctx, tc, out, x, m)
    nc.compile()
    _CACHE["nc"] = nc
    return nc


def _in_maps(output: np.ndarray, masks: np.ndarray):
    output = np.ascontiguousarray(output, dtype=np.float32)
    masks = np.ascontiguousarray(masks, dtype=np.float32)
    return [
        {"x": output[b].reshape(P, COLS), "m": masks[b].reshape(P, COLS)}
        for b in range(N_CORES)
    ]


def _finish(cs: np.ndarray) -> np.float32:
    """Per-subject scalar tail (fp32, mirrors the reference ordering).

    cs: [C, 6] device output — columns (inter, mask_sum, x_sum) per class,
    then (inter, mask_sum, x_sum) of the shed block (class 3, surfaced in
    row 0 by the collapse).
    """
    cs = cs.astype(np.float32)
    inter, msum, xsum = cs[:, 0].copy(), cs[:, 1].copy(), cs[:, 2].copy()
    inter[3] += cs[0, 3]
    msum[3] += cs[0, 4]
    xsum[3] += cs[0, 5]
    w = np.float32(1.0) / (msum * msum + np.float32(EPS))
    total = xsum + msum
    nom = (w * inter).sum(dtype=np.float32)
    den = (w * total + np.float32(EPS)).sum(dtype=np.float32)
    return np.float32(1.0) - np.float32(2.0) * nom / den


def run_sharded(output: np.ndarray, masks: np.ndarray, **spmd_kwargs):
    """Run the SPMD kernel; returns (loss[1], BassKernelResults)."""
    nc = _build()
    res = run_bass_kernel_spmd(
        nc, _in_maps(output, masks), list(range(N_CORES)), **spmd_kwargs
    )
    per_subj = np.array(
        [_finish(res.results[b]["class_sums"]) for b in range(N_CORES)],
        dtype=np.float32,
    )
    loss = (per_subj.sum(dtype=np.float32) / np.float32(B)).reshape(1)
    return loss.astype(np.float32), res


def kernel(output: np.ndarray, masks: np.ndarray) -> np.ndarray:
    loss, _ = run_sharded(output, masks)
    return loss


# revision 12
# speedup vs baseline: 1.1886x; 1.0328x over previous
"""Multi-class 3D DICE loss on 8 Trainium2 NeuronCores.

Data-parallel over the subject (batch) axis: core b reduces subject b's
[C=4, 64, 128, 128] volumes to per-class (inter, mask_sum, out_sum) partial
sums; the host applies the ~10-flop DICE scalar tail and averages the 8
per-subject losses.

Per-core layout: each input tensor is viewed as [128, 32768] where
partition q = c*32 + p (class c in partition block [32c, 32c+32)).

Engine split per chunk: DVE scalar_tensor_tensor (inter partials), ACT
activation-Copy accum (x and m partials), PE collapses partition blocks
into per-class sums with one matmul at the end. DVE ~1 pass and ACT ~2
passes both stay ahead of the ~80 us DMA stream.

SDMA straggler mitigation: HWDGE splits an R-row transfer into R/b
blocks (b = smallest divisor of R that is >= R/16), assigned to engines
0..R/b-1; engine 15 is intermittently ~18% slower (observed 21.6 vs 26.3
GB/s, 4/5 runs). [126]-row transfers (14 engines x 9 rows, measured at
the full 27 GB/s per engine, unlike [120]/[64] shapes which halve) move
5376 of the 32768 columns without engines 14/15; full-128 chunks move
the rest. That leaves engine 15 with 8/9.64 ~= 0.82x the busiest
engines' bytes -- matching its slow rate, so its finish time never
binds. Rows 126/127 of the shed columns ride 4 one-row flat DMAs
(sprayed across all 16 engines). The [126]-chunks are interleaved
between full chunks so engine 15's transient lag stays bounded, and the
stream ENDS on a [126]-chunk so the final completion never waits on
engine 15.
"""

import os
import sys
from contextlib import ExitStack

import numpy as np

for _p in ("/opt/trn_rl_repo",):
    if _p not in sys.path and os.path.isdir(_p):
        sys.path.insert(0, _p)

import concourse.bass as bass  # noqa: E402
import concourse.tile as tile  # noqa: E402
from concourse import bacc, mybir  # noqa: E402
from concourse.bass_utils import run_bass_kernel_spmd  # noqa: E402

N_CORES = 8
B, C = 8, 4
SPATIAL = 64 * 128 * 128            # 1,048,576 per (subject, class)
P = 128                             # SBUF partitions = C * 32
COLS = (C * SPATIAL) // P           # 32768 elements per partition
EPS = 1e-7
F32 = mybir.dt.float32

# Full-height chunks (all 128 rows; engine 15 gets 8 rows of each).
FULL = [(0, 6144), (6144, 6144), (12288, 6144), (18432, 4096), (22528, 2048),
        (24576, 1536), (26112, 768), (26880, 512)]
# 126-row chunks (engines 0-13 only), descending so the compute tail is tiny.
SHED0, SHED_FD = 27392, 5376
T126 = [(27392, 2688), (30080, 1536), (31616, 768), (32384, 384)]
assert FULL[-1][0] + FULL[-1][1] == SHED0
assert T126[-1][0] + T126[-1][1] == COLS
assert sum(fd for _, fd in T126) == SHED_FD

# Accumulator column layout ([P, 40]), one column per compute chunk below.
# SBUF accumulator words (32 B = 8 fp32 cols) must not see concurrent
# writes from two engines; the single ordered memset at t0 plus
# one-engine-per-(row, word) accumulation keeps every word safe.
#   cols  0..11  inter, DVE         cols 16..26  msum chunks 0..10, ACT
#   col   12     shed inter, DVE    cols 27..38  xsum, ACT
#   col   15     msum last, DVE     col 39/40    shed msum/xsum, ACT
# The shed tile lives on partitions 0-1 (ACT cannot start at partition
# 126), so its sums surface in the class-0 row of the collapse; the host
# adds them back into class 3.
NCHUNK = 12
SHED_INTER, SHED_MSUM, SHED_XSUM = 12, 39, 40
MSUM_DVE = 15
MSUM0 = 16
XSUM0 = 27
ACC_COLS = 48
LAST = NCHUNK - 1


def _dice_body(ctx: ExitStack, tc: "tile.TileContext", out_ap, x_ap, m_ap):
    nc = tc.nc
    add = mybir.AluOpType.add
    mult = mybir.AluOpType.mult
    Copy = mybir.ActivationFunctionType.Copy

    consts = ctx.enter_context(tc.tile_pool(name="consts", bufs=1))
    xpool = ctx.enter_context(tc.tile_pool(name="xin", bufs=2))
    mpool = ctx.enter_context(tc.tile_pool(name="min", bufs=2))
    xt126 = ctx.enter_context(tc.tile_pool(name="xt126", bufs=2))
    mt126 = ctx.enter_context(tc.tile_pool(name="mt126", bufs=2))
    shedp = ctx.enter_context(tc.tile_pool(name="shed", bufs=1))
    small = ctx.enter_context(tc.tile_pool(name="small", bufs=1))
    psum = ctx.enter_context(tc.tile_pool(name="psum", bufs=1, space="PSUM"))

    # Block indicator: ind[q, c] = 1.0 iff q // 32 == c. lhsT for the
    # partition-block -> per-class collapse.
    ind = consts.tile([P, C], F32)
    nc.vector.memset(ind[:], 0.0)
    for c in range(C):
        nc.vector.memset(ind[c * 32 : (c + 1) * 32, c : c + 1], 1.0)

    # Per-chunk partial sums. Zeroed once so partial-height chunks leave
    # their uncovered rows (and pad columns) at exactly 0.
    acc = small.tile([P, ACC_COLS], F32)
    nc.vector.memset(acc[:], 0.0)
    # Engines must write their full elementwise result somewhere; stride-0
    # broadcast dummies avoid real [P, fd] scratch tiles (HW-verified).
    dve_dummy = small.tile([P, 1], F32)
    act_dummy = small.tile([P, 1], F32)
    act_dummy2 = small.tile([P, 1], F32)

    def compute(xt, mt, n, cols):
        """Partial sums over tiles xt, mt rows [0:n) into acc cols
        (inter, xsum, msum) — msum None means DVE tensor_reduce into
        MSUM_DVE (used for the final chunk so ACT and DVE finish in
        parallel right after the last bytes land)."""
        fd = xt.shape[-1]
        c_inter, c_xsum, c_msum = cols
        nc.vector.scalar_tensor_tensor(
            out=dve_dummy[0:n, 0:1].broadcast_to((n, fd)),
            in0=xt[0:n, :],
            scalar=1.0,
            in1=mt[0:n, :],
            op0=mult,
            op1=mult,
            accum_out=acc[0:n, c_inter : c_inter + 1],
        )
        nc.scalar.activation(
            out=act_dummy2[0:n, 0:1].broadcast_to((n, fd)),
            in_=xt[0:n, :],
            func=Copy,
            accum_out=acc[0:n, c_xsum : c_xsum + 1],
        )
        if c_msum is not None:
            nc.scalar.activation(
                out=act_dummy[0:n, 0:1].broadcast_to((n, fd)),
                in_=mt[0:n, :],
                func=Copy,
                accum_out=acc[0:n, c_msum : c_msum + 1],
            )
        else:
            nc.vector.tensor_reduce(
                acc[0:n, MSUM_DVE : MSUM_DVE + 1],
                mt[0:n, :],
                axis=mybir.AxisListType.X,
                op=add,
            )

    def chunk_cols(k):
        return (k, XSUM0 + k, MSUM0 + k if k < LAST else None)

    def full_chunk(off, fd, k):
        xt = xpool.tile([P, fd], F32, tag="xt")
        nc.sync.dma_start(out=xt[:], in_=x_ap[:, off : off + fd])
        mt = mpool.tile([P, fd], F32, tag="mt")
        nc.sync.dma_start(out=mt[:], in_=m_ap[:, off : off + fd])
        compute(xt, mt, P, chunk_cols(k))

    def t126_chunk(off, fd, k):
        xt = xt126.tile([126, fd], F32, tag="xt")
        nc.sync.dma_start(out=xt[:], in_=x_ap[0:126, off : off + fd])
        mt = mt126.tile([126, fd], F32, tag="mt")
        nc.sync.dma_start(out=mt[:], in_=m_ap[0:126, off : off + fd])
        compute(xt, mt, 126, chunk_cols(k))

    # Interleave so engine 15's transient lag stays bounded and the stream
    # ends on a [126]-chunk (ring order == issue order below).
    xs = shedp.tile([2, SHED_FD], F32, tag="xs")
    ms = shedp.tile([2, SHED_FD], F32, tag="ms")

    full_chunk(*FULL[0], 0)
    full_chunk(*FULL[1], 1)
    t126_chunk(*T126[0], 2)
    full_chunk(*FULL[2], 3)
    t126_chunk(*T126[1], 4)
    # Rows 126/127 of the shed columns: one flat one-row DMA per (row,
    # tensor), each sprayed evenly across all 16 SDMA engines. They land
    # on shed-tile rows 0-1.
    for p in (126, 127):
        i = p - 126
        nc.sync.dma_start(out=xs[i : i + 1, :], in_=x_ap[p : p + 1, SHED0:COLS])
        nc.sync.dma_start(out=ms[i : i + 1, :], in_=m_ap[p : p + 1, SHED0:COLS])
    compute(xs, ms, 2, (SHED_INTER, SHED_XSUM, SHED_MSUM))
    full_chunk(*FULL[3], 5)
    full_chunk(*FULL[4], 6)
    t126_chunk(*T126[2], 7)
    full_chunk(*FULL[5], 8)
    full_chunk(*FULL[6], 9)
    full_chunk(*FULL[7], 10)
    t126_chunk(*T126[3], 11)

    # Partition blocks -> per-(class, chunk) sums in one matmul, then three
    # exact-range PSUM reduces -> [4, 3] class sums (inter, msum, xsum).
    # The remaining ~10-flop scalar tail runs on the host during unshard.
    ps = psum.tile([C, ACC_COLS], F32)
    nc.tensor.matmul(out=ps[:], lhsT=ind[:], rhs=acc[:], start=True, stop=True)
    sums = small.tile([C, 6], F32)
    nc.vector.tensor_reduce(
        sums[:, 0:1], ps[:, 0:NCHUNK], axis=mybir.AxisListType.X, op=add
    )
    nc.vector.tensor_reduce(
        sums[:, 1:2], ps[:, MSUM_DVE : MSUM0 + LAST], axis=mybir.AxisListType.X, op=add
    )
    nc.vector.tensor_reduce(
        sums[:, 2:3], ps[:, XSUM0 : XSUM0 + NCHUNK], axis=mybir.AxisListType.X, op=add
    )
    # Shed sums (surface in the class-0 row; host adds them to class 3).
    for dst, src_col in ((3, SHED_INTER), (4, SHED_MSUM), (5, SHED_XSUM)):
        nc.vector.tensor_reduce(
            sums[:, dst : dst + 1],
            ps[:, src_col : src_col + 1],
            axis=mybir.AxisListType.X,
            op=add,
        )
    nc.sync.dma_start(out=out_ap, in_=sums[:])


_CACHE: dict[str, object] = {}


def _build():
    if "nc" in _CACHE:
        return _CACHE["nc"]
    nc = bacc.Bacc("TRN2", target_bir_lowering=False, debug=False)
    x = nc.dram_tensor("x", [P, COLS], F32, kind="ExternalInput").ap()
    m = nc.dram_tensor("m", [P, COLS], F32, kind="ExternalInput").ap()
    out = nc.dram_tensor("class_sums", [C, 6], F32, kind="ExternalOutput").ap()
    with tile.TileContext(nc) as tc:
        with ExitStack() as ctx:
            _dice_body(ctx, tc, out, x, m)
    nc.compile()
    _CACHE["nc"] = nc
    return nc


def _in_maps(output: np.ndarray, masks: np.ndarray):
    output = np.ascontiguousarray(output, dtype=np.float32)
    masks = np.ascontiguousarray(masks, dtype=np.float32)
    return [
        {"x": output[b].reshape(P, COLS), "m": masks[b].reshape(P, COLS)}
        for b in range(N_CORES)
    ]


def _finish(cs: np.ndarray) -> np.float32:
    """Per-subject scalar tail (fp32, mirrors the reference ordering).

    cs: [C, 6] device output — columns (inter, mask_sum, x_sum) per class,
    then (inter, mask_sum, x_sum) of the shed block (class 3, surfaced in
    row 0 by the collapse).
    """
    cs = cs.astype(np.float32)
    inter, msum, xsum = cs[:, 0].copy(), cs[:, 1].copy(), cs[:, 2].copy()
    inter[3] += cs[0, 3]
    msum[3] += cs[0, 4]
    xsum[3] += cs[0, 5]
    w = np.float32(1.0) / (msum * msum + np.float32(EPS))
    total = xsum + msum
    nom = (w * inter).sum(dtype=np.float32)
    den = (w * total + np.float32(EPS)).sum(dtype=np.float32)
    return np.float32(1.0) - np.float32(2.0) * nom / den


def run_sharded(output: np.ndarray, masks: np.ndarray, **spmd_kwargs):
    """Run the SPMD kernel; returns (loss[1], BassKernelResults)."""
    nc = _build()
    res = run_bass_kernel_spmd(
        nc, _in_maps(output, masks), list(range(N_CORES)), **spmd_kwargs
    )
    per_subj = np.array(
        [_finish(res.results[b]["class_sums"]) for b in range(N_CORES)],
        dtype=np.float32,
    )
    loss = (per_subj.sum(dtype=np.float32) / np.float32(B)).reshape(1)
    return loss.astype(np.float32), res


def kernel(output: np.ndarray, masks: np.ndarray) -> np.ndarray:
    loss, _ = run_sharded(output, masks)
    return loss


# revision 13
# speedup vs baseline: 1.2120x; 1.0197x over previous
"""Multi-class 3D DICE loss on 8 Trainium2 NeuronCores.

Data-parallel over the subject (batch) axis: core b reduces subject b's
[C=4, 64, 128, 128] volumes to per-class (inter, mask_sum, out_sum) partial
sums; the host applies the ~10-flop DICE scalar tail and averages the 8
per-subject losses.

Per-core layout: each input tensor is viewed as [128, 32768] where
partition q = c*32 + p (class c in partition block [32c, 32c+32)).
Per chunk (descending sizes, 4 MiB DMAs in steady state):
  - DVE  scalar_tensor_tensor: partial sums of output*masks   (inter)
  - ACT  activation(Copy, accum_out): partial sums of output
  - ACT  activation(Copy, accum_out): partial sums of masks
  - PE   collapses partition blocks into per-class sums with one matmul
Engine budget per 8 MiB chunk-pair (~19.5 us of DMA at ~430 GB/s):
DVE one pass ~8.7 us, ACT two passes ~14.2 us — both stay ahead of the
DMA stream, so the stream never stalls on buffer reuse. The last (tiny)
chunk splits its two plain sums across ACT (x) and DVE (m) so the
post-last-byte compute tail is ~1 us. Every tail chunk gets a dedicated
buffer so all DMAs are issued with no waits and queue on the ring early.

Note on bandwidth regimes (measured): with the sibling NeuronCores of
this 8-core job streaming concurrently, per-core DMA throttles from the
c. 433 GB/s fabric rate to c. 358 GB/s (HBM-per-NC limit) or to an
asymmetric variant where one SDMA engine runs c. 18% slow. Uniform
full-128-row chunks are the right shape for all regimes; partial-height
transfer tricks that shift bytes between engines only help the (rare)
asymmetric-clean case and hurt the throttled one.
"""

import os
import sys
from contextlib import ExitStack

import numpy as np

for _p in ("/opt/trn_rl_repo",):
    if _p not in sys.path and os.path.isdir(_p):
        sys.path.insert(0, _p)

import concourse.bass as bass  # noqa: E402
import concourse.tile as tile  # noqa: E402
from concourse import bacc, mybir  # noqa: E402
from concourse.bass_utils import run_bass_kernel_spmd  # noqa: E402

N_CORES = 8
B, C = 8, 4
SPATIAL = 64 * 128 * 128            # 1,048,576 per (subject, class)
P = 128                             # SBUF partitions = C * 32
COLS = (C * SPATIAL) // P           # 32768 elements per partition
# Descending chunk schedule: big DMAs (4 MiB) for bandwidth in the steady
# state, small chunks at the end so the post-last-byte compute tail is tiny.
CHUNKS = [8192, 8192, 8192, 4096, 2048, 1024, 512, 256, 256]
BIG_FD = 4096  # chunks >= this land in the big pools, the rest in tail pools
assert sum(CHUNKS) == COLS
NCHUNK = len(CHUNKS)
LAST = NCHUNK - 1
EPS = 1e-7
F32 = mybir.dt.float32

# Accumulator column layout ([P, 33]). SBUF accumulator words (32 B = 8
# fp32 cols) must each be written by a single engine — mixing engines
# within one word produced intermittent lost-update corruption on HW.
#   cols  0..8   inter,  DVE  (words 0-1)
#   col   15     msum of last chunk, DVE tensor_reduce (word 1, DVE-owned)
#   cols 16..23  msum of chunks 0..7, ACT (word 2)
#   cols 24..32  xsum, ACT (words 3-4)
# Cols 9..14 are memset-0 padding (DVE-owned word); the final reduces read
# exact ranges so pad columns never contaminate a result.
INTER0 = 0
MSUM_DVE = 15
MSUM0 = 16
XSUM0 = 24
ACC_COLS = 33


def _dice_body(ctx: ExitStack, tc: "tile.TileContext", out_ap, x_ap, m_ap):
    nc = tc.nc
    add = mybir.AluOpType.add
    mult = mybir.AluOpType.mult
    Copy = mybir.ActivationFunctionType.Copy

    consts = ctx.enter_context(tc.tile_pool(name="consts", bufs=1))
    xpool = ctx.enter_context(tc.tile_pool(name="xin", bufs=2))
    mpool = ctx.enter_context(tc.tile_pool(name="min", bufs=2))
    # One dedicated pool per tail (chunk, tensor): no buffer reuse, so tail
    # DMAs issue with no waits; slots are exact-sized (pool slots are all
    # max-tile-sized, so one variable-size pool would waste SBUF).
    tails = {
        (j, t): ctx.enter_context(tc.tile_pool(name=f"{t}tail{j}", bufs=1))
        for j, fd in enumerate(CHUNKS)
        if fd < BIG_FD
        for t in ("x", "m")
    }
    small = ctx.enter_context(tc.tile_pool(name="small", bufs=1))
    psum = ctx.enter_context(tc.tile_pool(name="psum", bufs=1, space="PSUM"))

    # Block indicator: ind[q, c] = 1.0 iff q // 32 == c. lhsT for the
    # partition-block -> per-class collapse.
    ind = consts.tile([P, C], F32)
    nc.vector.memset(ind[:], 0.0)
    for c in range(C):
        nc.vector.memset(ind[c * 32 : (c + 1) * 32, c : c + 1], 1.0)

    # Per-chunk partial sums (see layout above); no cross-chunk deps.
    acc = small.tile([P, ACC_COLS], F32)
    nc.vector.memset(acc[:, 9:15], 0.0)
    # Engines must write their full elementwise result somewhere; stride-0
    # broadcast dummies avoid real [P, fd] scratch tiles (HW-verified).
    dve_dummy = small.tile([P, 1], F32)
    act_dummy = small.tile([P, 1], F32)
    act_dummy2 = small.tile([P, 1], F32)

    off = 0
    for j, fd in enumerate(CHUNKS):
        big = fd >= BIG_FD
        xt = (xpool if big else tails[(j, "x")]).tile([P, fd], F32, tag="xt")
        nc.sync.dma_start(out=xt[:], in_=x_ap[:, off : off + fd])
        mt = (mpool if big else tails[(j, "m")]).tile([P, fd], F32, tag="mt")
        nc.sync.dma_start(out=mt[:], in_=m_ap[:, off : off + fd])
        off += fd

        # inter partials on DVE: out = (x*1)*m, accum = X-reduce(out).
        nc.vector.scalar_tensor_tensor(
            out=dve_dummy.broadcast_to((P, fd)),
            in0=xt[:],
            scalar=1.0,
            in1=mt[:],
            op0=mult,
            op1=mult,
            accum_out=acc[:, INTER0 + j : INTER0 + j + 1],
        )
        # x-sum on ACT (x's DMA lands before m's).
        nc.scalar.activation(
            out=act_dummy2.broadcast_to((P, fd)),
            in_=xt[:],
            func=Copy,
            accum_out=acc[:, XSUM0 + j : XSUM0 + j + 1],
        )
        if j < LAST:
            nc.scalar.activation(
                out=act_dummy.broadcast_to((P, fd)),
                in_=mt[:],
                func=Copy,
                accum_out=acc[:, MSUM0 + j : MSUM0 + j + 1],
            )
        else:
            # Last chunk: m-sum on DVE so ACT and DVE finish in parallel
            # right after the final bytes land.
            nc.vector.tensor_reduce(
                acc[:, MSUM_DVE : MSUM_DVE + 1],
                mt[:],
                axis=mybir.AxisListType.X,
                op=add,
            )

    # Partition blocks -> per-(class, chunk) sums in one matmul, then three
    # exact-range PSUM reduces -> [4, 3] class sums (inter, msum, xsum).
    # The remaining ~10-flop scalar tail runs on the host during unshard.
    ps = psum.tile([C, ACC_COLS], F32)
    nc.tensor.matmul(out=ps[:], lhsT=ind[:], rhs=acc[:], start=True, stop=True)
    sums = small.tile([C, 3], F32)
    nc.vector.tensor_reduce(
        sums[:, 0:1], ps[:, INTER0 : INTER0 + NCHUNK], axis=mybir.AxisListType.X, op=add
    )
    nc.vector.tensor_reduce(
        sums[:, 1:2], ps[:, MSUM_DVE : MSUM0 + LAST], axis=mybir.AxisListType.X, op=add
    )
    nc.vector.tensor_reduce(
        sums[:, 2:3], ps[:, XSUM0 : XSUM0 + NCHUNK], axis=mybir.AxisListType.X, op=add
    )
    nc.sync.dma_start(out=out_ap, in_=sums[:])


_CACHE: dict[str, object] = {}


def _build():
    if "nc" in _CACHE:
        return _CACHE["nc"]
    nc = bacc.Bacc("TRN2", target_bir_lowering=False, debug=False)
    x = nc.dram_tensor("x", [P, COLS], F32, kind="ExternalInput").ap()
    m = nc.dram_tensor("m", [P, COLS], F32, kind="ExternalInput").ap()
    out = nc.dram_tensor("class_sums", [C, 3], F32, kind="ExternalOutput").ap()
    with tile.TileContext(nc) as tc:
        with ExitStack() as ctx:
            _dice_body(ctx, tc, out, x, m)
    nc.compile()
    _CACHE["nc"] = nc
    return nc


def _in_maps(output: np.ndarray, masks: np.ndarray):
    output = np.ascontiguousarray(output, dtype=np.float32)
    masks = np.ascontiguousarray(masks, dtype=np.float32)
    return [
        {"x": output[b].reshape(P, COLS), "m": masks[b].reshape(P, COLS)}
        for b in range(N_CORES)
    ]


def _finish(cs: np.ndarray) -> np.float32:
    """Per-subject scalar tail (fp32, mirrors the reference ordering).

    cs: [C, 3] device output — columns (inter, mask_sum, x_sum) per class.
    """
    cs = cs.astype(np.float32)
    inter, msum, xsum = cs[:, 0], cs[:, 1], cs[:, 2]
    w = np.float32(1.0) / (msum * msum + np.float32(EPS))
    total = xsum + msum
    nom = (w * inter).sum(dtype=np.float32)
    den = (w * total + np.float32(EPS)).sum(dtype=np.float32)
    return np.float32(1.0) - np.float32(2.0) * nom / den


def run_sharded(output: np.ndarray, masks: np.ndarray, **spmd_kwargs):
    """Run the SPMD kernel; returns (loss[1], BassKernelResults)."""
    nc = _build()
    res = run_bass_kernel_spmd(
        nc, _in_maps(output, masks), list(range(N_CORES)), **spmd_kwargs
    )
    per_subj = np.array(
        [_finish(res.results[b]["class_sums"]) for b in range(N_CORES)],
        dtype=np.float32,
    )
    loss = (per_subj.sum(dtype=np.float32) / np.float32(B)).reshape(1)
    return loss.astype(np.float32), res


def kernel(output: np.ndarray, masks: np.ndarray) -> np.ndarray:
    loss, _ = run_sharded(output, masks)
    return loss
